# revision 1
# baseline (speedup 1.0000x reference)
"""MGU (minimal gated unit) Bass kernel for Trainium2, 8-core SPMD.

Problem: B=128, T=512, D=U=512 fp32.
    xf = x @ Wf + bf ; xh = x @ Wh + bh            (parallel over B,T)
    scan over t: f = sigmoid(xf_t + h @ Uf)
                 S = tanh(xh_t + (f*h) @ Uh)
                 h = (1-f)*h + f*S
Output: final h [B, U].

Sharding: data-parallel over B (16 rows/core), weights replicated.

Layout ("T-layout"): everything on-chip keeps U (or D) on the partition
axis, batch on the free axis, so the sequential recurrence needs no
per-step transposes:
  - h/f/S/g tiles: [128p, kt*16b] = [128, 64]   (kt = U/128 = 4)
  - U-weights stationary: lhsT tiles Uf[k*128:,:][:, m*128:] as [128,128]
  - per-step matmul zT[m] = sum_k Uf[k,m].T @ hT[k]  -> [128, 4*16] PSUM
All matmuls run in bf16 (fp32 PSUM accumulate); validated numerically at
~5e-3 max rel err vs the fp32 reference.
"""

import os
import numpy as np
import ml_dtypes

import concourse.bass as bass
import concourse.bacc as bacc
import concourse.mybir as mybir
from concourse import tile
from concourse.bass_utils import run_bass_kernel_spmd

B, T, D, U = 128, 512, 512, 512
NCORES = 8
BC = B // NCORES          # batch rows per core = 16
KT = D // 128             # 4 contraction tiles
MT = U // 128             # 4 output tiles
CHUNK = 32                # phase-1 time-chunk; N = CHUNK*BC = 512 per matmul
NCHUNK = T // CHUNK

BF16 = mybir.dt.bfloat16
F32 = mybir.dt.float32
NPBF16 = ml_dtypes.bfloat16
AF = mybir.ActivationFunctionType
ALU = mybir.AluOpType

_CACHE = {}
LAST_RESULTS = None  # test harness can read exec_time_ns / profile from here


def _build(t_steps: int):
    nc = bacc.Bacc("TRN2", target_bir_lowering=False, debug=False)

    x_d = nc.dram_tensor("xT", [KT, 128, T * BC], BF16, kind="ExternalInput")
    wf_d = nc.dram_tensor("WfT", [128, KT * U], BF16, kind="ExternalInput")
    wh_d = nc.dram_tensor("WhT", [128, KT * U], BF16, kind="ExternalInput")
    uf_d = nc.dram_tensor("UfT", [128, KT * U], BF16, kind="ExternalInput")
    uh_d = nc.dram_tensor("UhT", [128, KT * U], BF16, kind="ExternalInput")
    bf_d = nc.dram_tensor("bfT", [128, MT], F32, kind="ExternalInput")
    bh_d = nc.dram_tensor("bhT", [128, MT], F32, kind="ExternalInput")
    out_d = nc.dram_tensor("hT_out", [128, KT * BC], F32, kind="ExternalOutput")

    with tile.TileContext(nc) as tc:
        with (
            tc.tile_pool(name="const", bufs=1) as cpool,
            tc.tile_pool(name="xchunk", bufs=2) as xpool,
            tc.tile_pool(name="work", bufs=3) as wpool,
            tc.tile_pool(name="spsum", bufs=4, space="PSUM") as spsum,
        ):
            # ---- resident tensors ----
            wf_sb = cpool.tile([128, KT * U], BF16, tag="wf")
            wh_sb = cpool.tile([128, KT * U], BF16, tag="wh")
            uf_sb = cpool.tile([128, KT * U], BF16, tag="uf")
            uh_sb = cpool.tile([128, KT * U], BF16, tag="uh")
            bf_sb = cpool.tile([128, MT], F32, tag="bf")
            bh_sb = cpool.tile([128, MT], F32, tag="bh")
            xf_all = cpool.tile([128, T * MT * BC], BF16, tag="xfall")
            xh_all = cpool.tile([128, T * MT * BC], BF16, tag="xhall")

            nc.sync.dma_start(wf_sb[:], wf_d[:])
            nc.sync.dma_start(wh_sb[:], wh_d[:])
            nc.sync.dma_start(uf_sb[:], uf_d[:])
            nc.sync.dma_start(uh_sb[:], uh_d[:])
            nc.sync.dma_start(bf_sb[:], bf_d[:])
            nc.sync.dma_start(bh_sb[:], bh_d[:])

            # strided views of the projection buffers: [p, t, m, b]
            xf_v = xf_all[:].rearrange("p (t m b) -> p t m b", t=T, m=MT, b=BC)
            xh_v = xh_all[:].rearrange("p (t m b) -> p t m b", t=T, m=MT, b=BC)

            # ---- phase 1: xf = x@Wf + bf, xh = x@Wh + bh (chunked over T) ----
            with tc.tile_pool(name="ppsum", bufs=3, space="PSUM") as ppsum:
                for c in range(NCHUNK):
                    xc = xpool.tile([128, KT * CHUNK * BC], BF16, tag="xc")
                    for k in range(KT):
                        nc.sync.dma_start(
                            xc[:, k * CHUNK * BC:(k + 1) * CHUNK * BC],
                            x_d[k, :, c * CHUNK * BC:(c + 1) * CHUNK * BC],
                        )
                    for (w_sb, b_sb, dst) in ((wf_sb, bf_sb, xf_v), (wh_sb, bh_sb, xh_v)):
                        for m in range(MT):
                            ps = ppsum.tile([128, CHUNK * BC], F32, tag="pp")
                            for k in range(KT):
                                nc.tensor.matmul(
                                    ps[:],
                                    w_sb[:, k * U + m * 128: k * U + (m + 1) * 128],
                                    xc[:, k * CHUNK * BC:(k + 1) * CHUNK * BC],
                                    start=(k == 0), stop=(k == KT - 1),
                                )
                            nc.scalar.activation(
                                dst[:, c * CHUNK:(c + 1) * CHUNK, m, :],
                                ps[:].rearrange("p (t b) -> p t b", t=CHUNK, b=BC),
                                AF.Identity,
                                bias=b_sb[:, m:m + 1],
                            )

            # ---- phase 2: the sequential scan ----
            h = wpool.tile([128, KT * BC], BF16, tag="h")
            nc.vector.memset(h[:], 0.0)

            for t in range(t_steps):
                # zf = h @ Uf   (T-layout: zfT[m] = sum_k Uf[k,m].T @ hT[k])
                zf = spsum.tile([128, MT * BC], F32, tag="z")
                for m in range(MT):
                    for k in range(KT):
                        nc.tensor.matmul(
                            zf[:, m * BC:(m + 1) * BC],
                            uf_sb[:, k * U + m * 128: k * U + (m + 1) * 128],
                            h[:, k * BC:(k + 1) * BC],
                            start=(k == 0), stop=(k == KT - 1),
                        )
                uf_t = wpool.tile([128, MT * BC], F32, tag="uft")
                nc.vector.tensor_tensor(uf_t[:], zf[:], xf_all[:, t * MT * BC:(t + 1) * MT * BC], ALU.add)
                f = wpool.tile([128, MT * BC], F32, tag="f")
                nc.scalar.activation(f[:], uf_t[:], AF.Sigmoid)
                g = wpool.tile([128, MT * BC], BF16, tag="g")
                nc.vector.tensor_tensor(g[:], f[:], h[:], ALU.mult)

                # zh = g @ Uh
                zh = spsum.tile([128, MT * BC], F32, tag="z")
                for m in range(MT):
                    for k in range(KT):
                        nc.tensor.matmul(
                            zh[:, m * BC:(m + 1) * BC],
                            uh_sb[:, k * U + m * 128: k * U + (m + 1) * 128],
                            g[:, k * BC:(k + 1) * BC],
                            start=(k == 0), stop=(k == KT - 1),
                        )
                uh_t = wpool.tile([128, MT * BC], F32, tag="uht")
                nc.vector.tensor_tensor(uh_t[:], zh[:], xh_all[:, t * MT * BC:(t + 1) * MT * BC], ALU.add)
                s = wpool.tile([128, MT * BC], F32, tag="s")
                nc.scalar.activation(s[:], uh_t[:], AF.Tanh)

                # h' = h - g + f*S   (t2 = h - g is off the critical path)
                t2 = wpool.tile([128, MT * BC], F32, tag="t2")
                nc.vector.tensor_tensor(t2[:], h[:], g[:], ALU.subtract)
                t3 = wpool.tile([128, MT * BC], F32, tag="t3")
                nc.vector.tensor_tensor(t3[:], f[:], s[:], ALU.mult)
                last = (t == t_steps - 1)
                hn = wpool.tile([128, MT * BC], F32 if last else BF16, tag="hout" if last else "h")
                nc.vector.tensor_tensor(hn[:], t2[:], t3[:], ALU.add)
                h = hn

            nc.sync.dma_start(out_d[:], h[:])

    nc.compile()
    return nc


def _prep_weight_t(w):
    # [D, U] fp32 -> [128, KT*U] bf16 with [:, k*U+m] = w[k*128+p, m]
    return np.ascontiguousarray(
        w.reshape(KT, 128, U).transpose(1, 0, 2).reshape(128, KT * U)
    ).astype(NPBF16)


def kernel(x, Wf, Uf, bf, Wh, Uh, bh):
    global LAST_RESULTS
    x = np.asarray(x, dtype=np.float32)
    Wf = np.asarray(Wf, dtype=np.float32)
    Uf = np.asarray(Uf, dtype=np.float32)
    Wh = np.asarray(Wh, dtype=np.float32)
    Uh = np.asarray(Uh, dtype=np.float32)
    bf = np.asarray(bf, dtype=np.float32)
    bh = np.asarray(bh, dtype=np.float32)

    t_steps = int(os.environ.get("BASS_MGU_T", T))
    if t_steps not in _CACHE:
        _CACHE[t_steps] = _build(t_steps)
    nc = _CACHE[t_steps]

    wf_t = _prep_weight_t(Wf)
    wh_t = _prep_weight_t(Wh)
    uf_t = _prep_weight_t(Uf)
    uh_t = _prep_weight_t(Uh)
    bf_t = np.ascontiguousarray(bf.reshape(MT, 128).T).astype(np.float32)
    bh_t = np.ascontiguousarray(bh.reshape(MT, 128).T).astype(np.float32)

    in_maps = []
    for ci in range(NCORES):
        xc = x[ci * BC:(ci + 1) * BC]                       # [BC, T, D]
        xt = xc.transpose(2, 1, 0)                          # [D, T, BC]
        xt = np.ascontiguousarray(xt.reshape(KT, 128, T * BC)).astype(NPBF16)
        in_maps.append({
            "xT": xt, "WfT": wf_t, "WhT": wh_t, "UfT": uf_t, "UhT": uh_t,
            "bfT": bf_t, "bhT": bh_t,
        })

    trace = bool(int(os.environ.get("BASS_MGU_TRACE", "0")))
    res = run_bass_kernel_spmd(nc, in_maps, list(range(NCORES)), trace=trace)
    LAST_RESULTS = res

    out = np.empty((B, U), dtype=np.float32)
    for ci in range(NCORES):
        ho = np.asarray(res.results[ci]["hT_out"])          # [128, KT*BC]
        out[ci * BC:(ci + 1) * BC] = (
            ho.reshape(128, KT, BC).transpose(2, 1, 0).reshape(BC, U)
        )
    return out


# revision 2
# speedup vs baseline: 807.6663x; 807.6663x over previous
"""MGU (minimal gated unit) Bass kernel for Trainium2, 8-core SPMD.

Problem: B=128, T=512, D=U=512 fp32.
    xf = x @ Wf + bf ; xh = x @ Wh + bh            (parallel over B,T)
    scan over t: f = sigmoid(xf_t + h @ Uf)
                 S = tanh(xh_t + (f*h) @ Uh)
                 h = (1-f)*h + f*S
Output: final h [B, U].

Sharding: data-parallel over B (16 rows/core), weights replicated.

Layout ("T-layout"): everything on-chip keeps U (or D) on the partition
axis, batch on the free axis, so the sequential recurrence needs no
per-step transposes:
  - h/f/S/g tiles: [128p, kt*16b] = [128, 64]   (kt = U/128 = 4)
  - U-weights stationary: lhsT tiles Uf[k*128:,:][:, m*128:] as [128,128]
  - per-step matmul zT[m] = sum_k Uf[k,m].T @ hT[k]  -> [128, 4*16] PSUM
All matmuls run in bf16 (fp32 PSUM accumulate); validated numerically at
~5e-3 max rel err vs the fp32 reference.
"""

import os
import numpy as np
import ml_dtypes

import concourse.bass as bass
import concourse.bacc as bacc
import concourse.mybir as mybir
from concourse import tile
from concourse.bass_utils import run_bass_kernel_spmd

B, T, D, U = 128, 512, 512, 512
NCORES = 8
BC = B // NCORES          # batch rows per core = 16
KT = D // 128             # 4 contraction tiles
MT = U // 128             # 4 output tiles
CHUNK = 32                # phase-1 time-chunk; N = CHUNK*BC = 512 per matmul
NCHUNK = T // CHUNK

BF16 = mybir.dt.bfloat16
F32 = mybir.dt.float32
NPBF16 = ml_dtypes.bfloat16
AF = mybir.ActivationFunctionType
ALU = mybir.AluOpType

_CACHE = {}
LAST_RESULTS = None  # test harness can read exec_time_ns / profile from here


def _build(t_steps: int):
    nc = bacc.Bacc("TRN2", target_bir_lowering=False, debug=False)

    x_d = nc.dram_tensor("xT", [KT, 128, T * BC], BF16, kind="ExternalInput")
    wf_d = nc.dram_tensor("WfT", [128, KT * U], BF16, kind="ExternalInput")
    wh_d = nc.dram_tensor("WhT", [128, KT * U], BF16, kind="ExternalInput")
    uf_d = nc.dram_tensor("UfT", [128, KT * U], BF16, kind="ExternalInput")
    uh_d = nc.dram_tensor("UhT", [128, KT * U], BF16, kind="ExternalInput")
    bf_d = nc.dram_tensor("bfT", [128, MT], F32, kind="ExternalInput")
    bh_d = nc.dram_tensor("bhT", [128, MT], F32, kind="ExternalInput")
    out_d = nc.dram_tensor("hT_out", [128, KT * BC], F32, kind="ExternalOutput")

    with tile.TileContext(nc) as tc:
        with (
            tc.tile_pool(name="const", bufs=1) as cpool,
            tc.tile_pool(name="xchunk", bufs=2) as xpool,
            tc.tile_pool(name="work", bufs=3) as wpool,
            tc.tile_pool(name="spsum", bufs=4, space="PSUM") as spsum,
        ):
            # ---- resident tensors ----
            wf_sb = cpool.tile([128, KT * U], BF16, tag="wf")
            wh_sb = cpool.tile([128, KT * U], BF16, tag="wh")
            uf_sb = cpool.tile([128, KT * U], BF16, tag="uf")
            uh_sb = cpool.tile([128, KT * U], BF16, tag="uh")
            bf_sb = cpool.tile([128, MT], F32, tag="bf")
            bh_sb = cpool.tile([128, MT], F32, tag="bh")
            xf_all = cpool.tile([128, T * MT * BC], BF16, tag="xfall")
            xh_all = cpool.tile([128, T * MT * BC], BF16, tag="xhall")

            nc.sync.dma_start(wf_sb[:], wf_d[:])
            nc.sync.dma_start(wh_sb[:], wh_d[:])
            nc.sync.dma_start(uf_sb[:], uf_d[:])
            nc.sync.dma_start(uh_sb[:], uh_d[:])
            nc.sync.dma_start(bf_sb[:], bf_d[:])
            nc.sync.dma_start(bh_sb[:], bh_d[:])

            # strided views of the projection buffers: [p, t, m, b]
            xf_v = xf_all[:].rearrange("p (t m b) -> p t m b", t=T, m=MT, b=BC)
            xh_v = xh_all[:].rearrange("p (t m b) -> p t m b", t=T, m=MT, b=BC)

            # ---- phase 1: xf = x@Wf + bf, xh = x@Wh + bh (chunked over T) ----
            with tc.tile_pool(name="ppsum", bufs=3, space="PSUM") as ppsum:
                for c in range(NCHUNK):
                    xc = xpool.tile([128, KT * CHUNK * BC], BF16, tag="xc")
                    for k in range(KT):
                        nc.sync.dma_start(
                            xc[:, k * CHUNK * BC:(k + 1) * CHUNK * BC],
                            x_d[k, :, c * CHUNK * BC:(c + 1) * CHUNK * BC],
                        )
                    for (w_sb, b_sb, dst) in ((wf_sb, bf_sb, xf_v), (wh_sb, bh_sb, xh_v)):
                        for m in range(MT):
                            ps = ppsum.tile([128, CHUNK * BC], F32, tag="pp")
                            for k in range(KT):
                                nc.tensor.matmul(
                                    ps[:],
                                    w_sb[:, k * U + m * 128: k * U + (m + 1) * 128],
                                    xc[:, k * CHUNK * BC:(k + 1) * CHUNK * BC],
                                    start=(k == 0), stop=(k == KT - 1),
                                )
                            nc.scalar.activation(
                                dst[:, c * CHUNK:(c + 1) * CHUNK, m, :],
                                ps[:].rearrange("p (t b) -> p t b", t=CHUNK, b=BC),
                                AF.Identity,
                                bias=b_sb[:, m:m + 1],
                            )

            # ---- phase 2: the sequential scan ----
            h = wpool.tile([128, KT * BC], BF16, tag="h")
            nc.vector.memset(h[:], 0.0)

            for t in range(t_steps):
                # zf = h @ Uf   (T-layout: zfT[m] = sum_k Uf[k,m].T @ hT[k])
                zf = spsum.tile([128, MT * BC], F32, tag="z")
                for m in range(MT):
                    for k in range(KT):
                        nc.tensor.matmul(
                            zf[:, m * BC:(m + 1) * BC],
                            uf_sb[:, k * U + m * 128: k * U + (m + 1) * 128],
                            h[:, k * BC:(k + 1) * BC],
                            start=(k == 0), stop=(k == KT - 1),
                        )
                uf_t = wpool.tile([128, MT * BC], F32, tag="uft")
                nc.vector.tensor_tensor(uf_t[:], zf[:], xf_all[:, t * MT * BC:(t + 1) * MT * BC], ALU.add)
                f = wpool.tile([128, MT * BC], F32, tag="f")
                nc.scalar.activation(f[:], uf_t[:], AF.Sigmoid)
                g = wpool.tile([128, MT * BC], BF16, tag="g")
                nc.vector.tensor_tensor(g[:], f[:], h[:], ALU.mult)

                # zh = g @ Uh
                zh = spsum.tile([128, MT * BC], F32, tag="z")
                for m in range(MT):
                    for k in range(KT):
                        nc.tensor.matmul(
                            zh[:, m * BC:(m + 1) * BC],
                            uh_sb[:, k * U + m * 128: k * U + (m + 1) * 128],
                            g[:, k * BC:(k + 1) * BC],
                            start=(k == 0), stop=(k == KT - 1),
                        )
                uh_t = wpool.tile([128, MT * BC], F32, tag="uht")
                nc.vector.tensor_tensor(uh_t[:], zh[:], xh_all[:, t * MT * BC:(t + 1) * MT * BC], ALU.add)
                s = wpool.tile([128, MT * BC], F32, tag="s")
                nc.scalar.activation(s[:], uh_t[:], AF.Tanh)

                # h' = h - g + f*S   (t2 = h - g is off the critical path)
                t2 = wpool.tile([128, MT * BC], F32, tag="t2")
                nc.vector.tensor_tensor(t2[:], h[:], g[:], ALU.subtract)
                t3 = wpool.tile([128, MT * BC], F32, tag="t3")
                nc.vector.tensor_tensor(t3[:], f[:], s[:], ALU.mult)
                last = (t == t_steps - 1)
                hn = wpool.tile([128, MT * BC], F32 if last else BF16, tag="hout" if last else "h")
                nc.vector.tensor_tensor(hn[:], t2[:], t3[:], ALU.add)
                h = hn

            nc.sync.dma_start(out_d[:], h[:])

    nc.compile()
    return nc


def _prep_weight_t(w):
    # [D, U] fp32 -> [128, KT*U] bf16 with [:, k*U+m] = w[k*128+p, m]
    return np.ascontiguousarray(
        w.reshape(KT, 128, U).transpose(1, 0, 2).reshape(128, KT * U)
    ).astype(NPBF16)


def kernel(x, Wf, Uf, bf, Wh, Uh, bh):
    global LAST_RESULTS
    x = np.asarray(x, dtype=np.float32)
    Wf = np.asarray(Wf, dtype=np.float32)
    Uf = np.asarray(Uf, dtype=np.float32)
    Wh = np.asarray(Wh, dtype=np.float32)
    Uh = np.asarray(Uh, dtype=np.float32)
    bf = np.asarray(bf, dtype=np.float32)
    bh = np.asarray(bh, dtype=np.float32)

    t_steps = int(os.environ.get("BASS_MGU_T", T))
    if t_steps not in _CACHE:
        _CACHE[t_steps] = _build(t_steps)
    nc = _CACHE[t_steps]

    wf_t = _prep_weight_t(Wf)
    wh_t = _prep_weight_t(Wh)
    uf_t = _prep_weight_t(Uf)
    uh_t = _prep_weight_t(Uh)
    bf_t = np.ascontiguousarray(bf.reshape(MT, 128).T).astype(np.float32)
    bh_t = np.ascontiguousarray(bh.reshape(MT, 128).T).astype(np.float32)

    in_maps = []
    for ci in range(NCORES):
        xc = x[ci * BC:(ci + 1) * BC]                       # [BC, T, D]
        xt = xc.transpose(2, 1, 0)                          # [D, T, BC]
        xt = np.ascontiguousarray(xt.reshape(KT, 128, T * BC)).astype(NPBF16)
        in_maps.append({
            "xT": xt, "WfT": wf_t, "WhT": wh_t, "UfT": uf_t, "UhT": uh_t,
            "bfT": bf_t, "bhT": bh_t,
        })

    trace = bool(int(os.environ.get("BASS_MGU_TRACE", "0")))
    kw = {}
    if trace and os.environ.get("BASS_TRACE_DIR"):
        kw["tmpdir"] = os.environ["BASS_TRACE_DIR"]
    res = run_bass_kernel_spmd(nc, in_maps, list(range(NCORES)), trace=trace, **kw)
    LAST_RESULTS = res

    out = np.empty((B, U), dtype=np.float32)
    for ci in range(NCORES):
        ho = np.asarray(res.results[ci]["hT_out"])          # [128, KT*BC]
        out[ci * BC:(ci + 1) * BC] = (
            ho.reshape(128, KT, BC).transpose(2, 1, 0).reshape(BC, U)
        )
    return out


# revision 6
# speedup vs baseline: 989.1555x; 1.2247x over previous
"""MGU (minimal gated unit) Bass kernel for Trainium2, 8-core SPMD.

Problem: B=128, T=512, D=U=512 fp32.
    xf = x @ Wf + bf ; xh = x @ Wh + bh            (parallel over B,T)
    scan over t: f = sigmoid(xf_t + h @ Uf)
                 S = tanh(xh_t + (f*h) @ Uh)
                 h = (1-f)*h + f*S
Output: final h [B, U].

Sharding: data-parallel over B (16 rows/core), weights replicated.

Layout ("T-layout"): U (or D) stays on the partition axis, batch on the
free axis, so the sequential recurrence needs no per-step transposes:
  - h/f/S/g tiles: [128p, kt*16b] = [128, 64]   (kt = U/128 = 4)
  - per-step matmul zT[m] = sum_k Uf[k,m].T @ hT[k] -> [128, 4*16] PSUM
All matmuls in bf16 (fp32 PSUM accumulate); ~5e-3 max rel err.

Perf structure:
  - xf_t/xh_t are seeded into the PSUM accumulator by an identity-weight
    matmul (sets has_written), removing the DVE adds from the serial
    chain; sigmoid/tanh read PSUM directly.
  - phase-1 projection matmuls are interleaved into the scan's PE gaps
    (one m-group every 4 steps), hiding phase 1 entirely and keeping the
    PE HAM-unthrottled at 2.4 GHz.
  - t2 = h - g runs on the idle GpSimd engine, off the critical chain.
"""

import os
import numpy as np
import ml_dtypes

import concourse.bass as bass
import concourse.bacc as bacc
import concourse.mybir as mybir
from concourse import tile
from concourse.bass_utils import run_bass_kernel_spmd

B, T, D, U = 128, 512, 512, 512
NCORES = 8
BC = B // NCORES          # batch rows per core = 16
KT = D // 128             # 4 contraction tiles
MT = U // 128             # 4 output tiles
CHUNK = 32                # phase-1 time-chunk; N = CHUNK*BC = 512 per matmul
GW = MT * BC              # scan tile width = 64

BF16 = mybir.dt.bfloat16
F32 = mybir.dt.float32
NPBF16 = ml_dtypes.bfloat16
AF = mybir.ActivationFunctionType
ALU = mybir.AluOpType

_CACHE = {}
LAST_RESULTS = None  # test harness reads exec_time_ns / profile from here


def _build(t_steps: int):
    nc = bacc.Bacc("TRN2", target_bir_lowering=False, debug=False)
    nchunk = (t_steps + CHUNK - 1) // CHUNK

    x_d = nc.dram_tensor("xT", [KT, 128, T * BC], BF16, kind="ExternalInput")
    wf_d = nc.dram_tensor("WfT", [128, KT * U], BF16, kind="ExternalInput")
    wh_d = nc.dram_tensor("WhT", [128, KT * U], BF16, kind="ExternalInput")
    uf_d = nc.dram_tensor("UfT", [128, KT * U], BF16, kind="ExternalInput")
    uh_d = nc.dram_tensor("UhT", [128, KT * U], BF16, kind="ExternalInput")
    bf_d = nc.dram_tensor("bfT", [128, MT], F32, kind="ExternalInput")
    bh_d = nc.dram_tensor("bhT", [128, MT], F32, kind="ExternalInput")
    eye_d = nc.dram_tensor("eye", [128, 128], BF16, kind="ExternalInput")
    out_d = nc.dram_tensor("hT_out", [128, KT * BC], F32, kind="ExternalOutput")

    with tile.TileContext(nc) as tc:
        with (
            tc.tile_pool(name="const", bufs=1) as cpool,
            tc.tile_pool(name="xchunk", bufs=3) as xpool,
            tc.tile_pool(name="proj", bufs=16) as projpool,
            tc.tile_pool(name="work", bufs=3) as wpool,
            tc.tile_pool(name="spsum", bufs=4, space="PSUM") as spsum,
            tc.tile_pool(name="ppsum", bufs=2, space="PSUM") as ppsum,
        ):
            # ---- resident tensors ----
            wf_sb = cpool.tile([128, KT * U], BF16, tag="wf")
            wh_sb = cpool.tile([128, KT * U], BF16, tag="wh")
            uf_sb = cpool.tile([128, KT * U], BF16, tag="uf")
            uh_sb = cpool.tile([128, KT * U], BF16, tag="uh")
            bf_sb = cpool.tile([128, MT], F32, tag="bf")
            bh_sb = cpool.tile([128, MT], F32, tag="bh")
            eye_sb = cpool.tile([128, 128], BF16, tag="eye")

            nc.sync.dma_start(wf_sb[:], wf_d[:])
            nc.sync.dma_start(wh_sb[:], wh_d[:])
            nc.sync.dma_start(uf_sb[:], uf_d[:])
            nc.sync.dma_start(uh_sb[:], uh_d[:])
            nc.sync.dma_start(bf_sb[:], bf_d[:])
            nc.sync.dma_start(bh_sb[:], bh_d[:])
            nc.sync.dma_start(eye_sb[:], eye_d[:])

            # per-chunk projection tiles (bf16): free = (t_local, m, b)
            xf_c = [None] * nchunk
            xh_c = [None] * nchunk
            xc_c = [None] * nchunk

            def emit_chunk_dma(c):
                xc = xpool.tile([128, KT * CHUNK * BC], BF16, tag="xc")
                for k in range(KT):
                    nc.sync.dma_start(
                        xc[:, k * CHUNK * BC:(k + 1) * CHUNK * BC],
                        x_d[k, :, c * CHUNK * BC:(c + 1) * CHUNK * BC],
                    )
                xc_c[c] = xc
                xf_c[c] = projpool.tile([128, CHUNK * GW], BF16, tag="xfc", name=f"xfc{c}")
                xh_c[c] = projpool.tile([128, CHUNK * GW], BF16, tag="xhc", name=f"xhc{c}")

            def emit_proj_group(c, gi):
                """One (gate, m) projection group of chunk c: 4 matmuls + ACT copy."""
                gate, m = divmod(gi, MT)
                w_sb, b_sb, dst = ((wf_sb, bf_sb, xf_c[c]), (wh_sb, bh_sb, xh_c[c]))[gate]
                xc = xc_c[c]
                ps = ppsum.tile([128, CHUNK * BC], F32, tag="pp")
                for k in range(KT):
                    nc.tensor.matmul(
                        ps[:],
                        w_sb[:, k * U + m * 128: k * U + (m + 1) * 128],
                        xc[:, k * CHUNK * BC:(k + 1) * CHUNK * BC],
                        start=(k == 0), stop=(k == KT - 1),
                    )
                dv = dst[:].rearrange("p (t m b) -> p t m b", t=CHUNK, m=MT, b=BC)
                nc.scalar.activation(
                    dv[:, :, m, :],
                    ps[:].rearrange("p (t b) -> p t b", t=CHUNK, b=BC),
                    AF.Identity,
                    bias=b_sb[:, m:m + 1],
                )

            # prologue: first two chunks fully
            for c in range(min(2, nchunk)):
                emit_chunk_dma(c)
                for gi in range(2 * MT):
                    emit_proj_group(c, gi)

            # ---- the sequential scan, with projection work interleaved ----
            h = wpool.tile([128, GW], BF16, tag="h")
            nc.vector.memset(h[:], 0.0)

            def gate_matmuls(z, u_sb, rhs, xsrc):
                # seed z with x-projection via identity weights, then accumulate
                nc.tensor.matmul(z[:], eye_sb[:], xsrc, start=True, stop=False,
                                 skip_group_check=True)
                for m in range(MT):
                    for k in range(KT):
                        nc.tensor.matmul(
                            z[:, m * BC:(m + 1) * BC],
                            u_sb[:, k * U + m * 128: k * U + (m + 1) * 128],
                            rhs[:, k * BC:(k + 1) * BC],
                            start=False, stop=(m == MT - 1 and k == KT - 1),
                            skip_group_check=True,
                        )

            for t in range(t_steps):
                c, tl = divmod(t, CHUNK)
                # interleave next-next chunk's projection work into PE gaps
                nxt = c + 2
                if nxt < nchunk:
                    if tl == 0:
                        emit_chunk_dma(nxt)
                    if tl % 4 == 1:
                        emit_proj_group(nxt, tl // 4)

                zf = spsum.tile([128, GW], F32, tag="z")
                gate_matmuls(zf, uf_sb, h, xf_c[c][:, tl * GW:(tl + 1) * GW])
                f = wpool.tile([128, GW], F32, tag="f")
                nc.scalar.activation(f[:], zf[:], AF.Sigmoid)
                g = wpool.tile([128, GW], BF16, tag="g")
                nc.vector.tensor_tensor(g[:], f[:], h[:], ALU.mult)
                t2 = wpool.tile([128, GW], F32, tag="t2")
                nc.gpsimd.tensor_tensor(t2[:], h[:], g[:], ALU.subtract)

                zh = spsum.tile([128, GW], F32, tag="z")
                gate_matmuls(zh, uh_sb, g, xh_c[c][:, tl * GW:(tl + 1) * GW])
                s = wpool.tile([128, GW], F32, tag="s")
                nc.scalar.activation(s[:], zh[:], AF.Tanh)

                # h' = t2 + f*S
                t3 = wpool.tile([128, GW], F32, tag="t3")
                nc.vector.tensor_tensor(t3[:], f[:], s[:], ALU.mult)
                last = (t == t_steps - 1)
                hn = wpool.tile([128, GW], F32 if last else BF16, tag="hout" if last else "h")
                nc.vector.tensor_tensor(hn[:], t2[:], t3[:], ALU.add)
                h = hn

            nc.sync.dma_start(out_d[:], h[:])

    nc.compile()
    return nc


def _prep_weight_t(w):
    # [D, U] fp32 -> [128, KT*U] bf16 with [:, k*U+m] = w[k*128+p, m]
    return np.ascontiguousarray(
        w.reshape(KT, 128, U).transpose(1, 0, 2).reshape(128, KT * U)
    ).astype(NPBF16)


def kernel(x, Wf, Uf, bf, Wh, Uh, bh):
    global LAST_RESULTS
    x = np.asarray(x, dtype=np.float32)
    Wf = np.asarray(Wf, dtype=np.float32)
    Uf = np.asarray(Uf, dtype=np.float32)
    Wh = np.asarray(Wh, dtype=np.float32)
    Uh = np.asarray(Uh, dtype=np.float32)
    bf = np.asarray(bf, dtype=np.float32)
    bh = np.asarray(bh, dtype=np.float32)

    t_steps = int(os.environ.get("BASS_MGU_T", T))
    if t_steps not in _CACHE:
        _CACHE[t_steps] = _build(t_steps)
    nc = _CACHE[t_steps]

    wf_t = _prep_weight_t(Wf)
    wh_t = _prep_weight_t(Wh)
    uf_t = _prep_weight_t(Uf)
    uh_t = _prep_weight_t(Uh)
    bf_t = np.ascontiguousarray(bf.reshape(MT, 128).T).astype(np.float32)
    bh_t = np.ascontiguousarray(bh.reshape(MT, 128).T).astype(np.float32)
    eye = np.eye(128, dtype=np.float32).astype(NPBF16)

    in_maps = []
    for ci in range(NCORES):
        xc = x[ci * BC:(ci + 1) * BC]                       # [BC, T, D]
        xt = xc.transpose(2, 1, 0)                          # [D, T, BC]
        xt = np.ascontiguousarray(xt.reshape(KT, 128, T * BC)).astype(NPBF16)
        in_maps.append({
            "xT": xt, "WfT": wf_t, "WhT": wh_t, "UfT": uf_t, "UhT": uh_t,
            "bfT": bf_t, "bhT": bh_t, "eye": eye,
        })

    trace = bool(int(os.environ.get("BASS_MGU_TRACE", "0")))
    kw = {}
    if trace and os.environ.get("BASS_TRACE_DIR"):
        kw["tmpdir"] = os.environ["BASS_TRACE_DIR"]
    res = run_bass_kernel_spmd(nc, in_maps, list(range(NCORES)), trace=trace, **kw)
    LAST_RESULTS = res

    out = np.empty((B, U), dtype=np.float32)
    for ci in range(NCORES):
        ho = np.asarray(res.results[ci]["hT_out"])          # [128, KT*BC]
        out[ci * BC:(ci + 1) * BC] = (
            ho.reshape(128, KT, BC).transpose(2, 1, 0).reshape(BC, U)
        )
    return out
